# revision 7
# baseline (speedup 1.0000x reference)
"""Trainium2 Bass kernel for KANPolyLayer:
    y[b,o] = sum_{i,p} x[b,i]^p * coeffs[o,i,p] + bias[o],  p = 0..4

Math: y = sum_{p=1..4} (x^p) @ C_p^T + biascol, with C_p = coeffs[:, :, p]
and biascol = bias + colsum(C_0) folded on host (plane 0 is x-independent).

All four planes run as fp32r (FP22) matmuls — measured at ~227 ns per
128x128x512 MM, the fastest per-MAC rate on this part (bf16 pays a
separate LDWEIGHTS ~259 ns; fp8 DoubleRow streams its two subtiles
sequentially, so it is not faster either).

Schedule per core: x tiles stream on the Sync engine's DMA path while
coeff tiles stream on GpSimd's, so the first k-plane's inputs land ~2x
sooner and DMA issue never serializes behind one engine.  Powers are
computed on-chip (ScalarE square + VectorE muls).  All 8 output groups
(4 o-tiles x 2 b-halves) accumulate concurrently in 8 PSUM banks.
Warmup matmuls spin the HAM clock-gate to 2.4 GHz during the DMA head.
The last NTAIL k-planes are emitted group-contiguous (two groups
interleaved per block) so bias-add + output DMA overlap the stream.

Sharding (8 cores): 4 batch groups x 2 out-dim groups.
  core c -> (bg, og) = (c // 2, c % 2)
Each core computes a disjoint (512 x 1024) block of yT; host gathers.
"""

from contextlib import ExitStack

import numpy as np

import concourse.bacc as bacc
import concourse.bass as bass
import concourse.mybir as mybir
import concourse.tile as tile
from concourse.bass_utils import run_bass_kernel_spmd

F32 = mybir.dt.float32
F32R = mybir.dt.float32r

B, I, O = 4096, 1024, 1024  # batch, in_dim, out_dim
BW, OW = 4, 2               # batch groups x out-dim groups (8 cores)
BS, OS = B // BW, O // OW   # per-core batch (1024) and out (512)
NK = I // 128               # contraction tiles (8)
NT = OS // 128              # o-tiles (4)
NH = BS // 512              # b-halves (2)
NTAIL = 2                   # trailing k-planes emitted group-contiguous
WARMUP = 14                 # PE warmup matmuls (HAM un-throttle)

_CACHE: dict = {}


def _build():
    nc = bacc.Bacc("TRN2", target_bir_lowering=False, debug=False, num_devices=8)

    xt = nc.dram_tensor("xt", [I, BS], F32, kind="ExternalInput")        # [i, b]
    ct = nc.dram_tensor("ct", [4, I, OS], F32, kind="ExternalInput")     # [p-1, i, o]
    bc = nc.dram_tensor("bc", [OS, 1], F32, kind="ExternalInput")
    yt = nc.dram_tensor("yt", [OS, BS], F32, kind="ExternalOutput")      # [o, b]

    with tile.TileContext(nc) as tc, ExitStack() as ctx:
        cons = ctx.enter_context(tc.tile_pool(name="cons", bufs=1))
        cpool = ctx.enter_context(tc.tile_pool(name="coef", bufs=12))
        ppool = ctx.enter_context(tc.tile_pool(name="pow", bufs=1))
        opool = ctx.enter_context(tc.tile_pool(name="out", bufs=3))
        pspool = ctx.enter_context(
            tc.tile_pool(name="ps", bufs=8, space=bass.MemorySpace.PSUM)
        )

        ps = {}
        for ot in range(NT):
            for h in range(NH):
                ps[(ot, h)] = pspool.tile(
                    [128, 512], F32, tag="ps", name=f"ps_{ot}_{h}"
                )

        # PE warmup on a memset tile while the first input DMAs fly
        wr = cons.tile([128, 256], mybir.dt.bfloat16)
        nc.vector.memset(wr[:], 0.0)
        for w in range(WARMUP):
            nc.tensor.matmul(
                ps[(0, 0)][:, 0:256], wr[:, 0:128], wr[:], start=True, stop=True,
                skip_group_check=True,
            )

        biascol = cons.tile([128, NT], F32)

        cpts = {}
        pows = {}
        for k in range(NK):
            ksl = slice(k * 128, (k + 1) * 128)
            # x on Sync's DMA path; coeffs on GpSimd's
            xk = ppool.tile([128, BS], F32R, tag=f"x_{k}", name=f"x_{k}")
            if k == 0:
                for h in range(NH):
                    nc.sync.dma_start(
                        xk[:, h * 512:(h + 1) * 512],
                        xt[ksl, h * 512:(h + 1) * 512].bitcast(F32R),
                    )
            else:
                nc.sync.dma_start(xk[:], xt[ksl, :].bitcast(F32R))
            for p in range(1, 5):
                cpt = cpool.tile([128, OS], F32R, tag="cp", name=f"cpt_{k}_{p}")
                nc.gpsimd.dma_start(cpt[:], ct[p - 1, ksl, :].bitcast(F32R))
                cpts[(k, p)] = cpt
            if k == 1:
                # bias column (host-folded bias + colsum(C0)); only needed
                # at the end, so it rides behind the first coeff tiles
                for ot in range(NT):
                    nc.gpsimd.dma_start(
                        biascol[:, ot:ot + 1], bc[ot * 128:(ot + 1) * 128, :]
                    )

            # powers: p2 = x^2 (ScalarE), p3/p4 (VectorE); first k-planes
            # split per-half so the matmul stream starts sooner
            p2 = ppool.tile([128, BS], F32R, tag=f"p2_{k}", name=f"p2_{k}")
            p3 = ppool.tile([128, BS], F32R, tag=f"p3_{k}", name=f"p3_{k}")
            p4 = ppool.tile([128, BS], F32R, tag=f"p4_{k}", name=f"p4_{k}")
            halves = (slice(0, 512), slice(512, 1024)) if k < 2 else (slice(0, 1024),)
            for sl in halves:
                nc.scalar.square(p2[:, sl], xk[:, sl])
                nc.vector.tensor_mul(p3[:, sl], p2[:, sl], xk[:, sl])
                nc.vector.tensor_mul(p4[:, sl], p2[:, sl], p2[:, sl])
            pows[k] = {1: xk, 2: p2, 3: p3, 4: p4}

            if k < NK - NTAIL:
                for p in range(1, 5):
                    for ot in range(NT):
                        osl = slice(ot * 128, (ot + 1) * 128)
                        for h in range(NH):
                            sl = slice(h * 512, (h + 1) * 512)
                            nc.tensor.matmul(
                                ps[(ot, h)],
                                cpts[(k, p)][:, osl],
                                pows[k][p][:, sl],
                                start=(k == 0 and p == 1),
                                stop=False,
                            )

        # trailing k-planes group-contiguous, two groups (the two b-halves
        # of one o-tile) interleaved per block: each block finishes ~3.6us
        # apart, so bias-add + output DMA overlap the matmul stream
        for ot in range(NT):
            osl = slice(ot * 128, (ot + 1) * 128)
            for k in range(NK - NTAIL, NK):
                for p in range(1, 5):
                    for h in range(NH):
                        sl = slice(h * 512, (h + 1) * 512)
                        nc.tensor.matmul(
                            ps[(ot, h)],
                            cpts[(k, p)][:, osl],
                            pows[k][p][:, sl],
                            start=False,
                            stop=(k == NK - 1 and p == 4),
                        )
            for h in range(NH):
                sl = slice(h * 512, (h + 1) * 512)
                o_sb = opool.tile([128, 512], F32, tag="o_sb", name=f"o_{ot}_{h}")
                nc.scalar.activation(
                    o_sb[:, 0:256],
                    ps[(ot, h)][:, 0:256],
                    mybir.ActivationFunctionType.Identity,
                    bias=biascol[:, ot:ot + 1],
                )
                nc.vector.tensor_scalar_add(
                    o_sb[:, 256:512], ps[(ot, h)][:, 256:512], biascol[:, ot:ot + 1]
                )
                nc.sync.dma_start(
                    yt[osl, h * 512:h * 512 + 256], o_sb[:, 0:256]
                )
                nc.sync.dma_start(
                    yt[osl, h * 512 + 256:(h + 1) * 512], o_sb[:, 256:512]
                )

    nc.compile()
    return nc


def _get_nc():
    if "nc" not in _CACHE:
        _CACHE["nc"] = _build()
    return _CACHE["nc"]


def _make_in_maps(x, coeffs, bias):
    x = np.asarray(x, dtype=np.float32)
    coeffs = np.asarray(coeffs, dtype=np.float32)
    bias = np.asarray(bias, dtype=np.float32)

    xts = [
        np.ascontiguousarray(x[bg * BS:(bg + 1) * BS, :].T) for bg in range(BW)
    ]
    cts = []
    bcs = []
    for og in range(OW):
        csl = coeffs[og * OS:(og + 1) * OS, :, :]  # [OS, I, 5]
        cts.append(np.ascontiguousarray(csl[:, :, 1:].transpose(2, 1, 0)))
        bcs.append(
            np.ascontiguousarray(
                (bias[0, og * OS:(og + 1) * OS] + csl[:, :, 0].sum(axis=1))
                .reshape(OS, 1)
            )
        )
    in_maps = []
    for c in range(BW * OW):
        bg, og = c // OW, c % OW
        in_maps.append({"xt": xts[bg], "ct": cts[og], "bc": bcs[og]})
    return in_maps


def _gather(results):
    y = np.empty((B, O), dtype=np.float32)
    for c, res in enumerate(results):
        bg, og = c // OW, c % OW
        y[bg * BS:(bg + 1) * BS, og * OS:(og + 1) * OS] = res["yt"].T
    return y


def run(x, coeffs, bias, trace=False, **trace_kwargs):
    nc = _get_nc()
    in_maps = _make_in_maps(x, coeffs, bias)
    br = run_bass_kernel_spmd(
        nc, in_maps, list(range(BW * OW)), trace=trace, **trace_kwargs
    )
    return _gather(br.results), br


def kernel(x, coeffs, bias):
    out, _ = run(x, coeffs, bias)
    return out
